# revision 10
# baseline (speedup 1.0000x reference)
"""BYOL loss kernel for Trainium2 (8 NeuronCores, SPMD data-parallel).

loss = 2 - 2 * mean_n( <x_n, t_n> / (||x_n|| * ||t_n||) )   over N=8192 rows, D=512.

v6 design (HW-measured op costs):
- Host casts f32 -> bf16, reshapes each core's [1024, 512] shards and
  interleaves x/t per 128-row block into ONE [128, 2*4096] DRAM tensor:
  row r = p*8 + slot on partition p; block i's x at free-offset 2i*512,
  t at (2i+1)*512. Every DMA chunk delivers complete (x,t) block pairs,
  so each DMA-completion semaphore unlocks all 3 products of its blocks
  (xt, xx, tt) for both compute engines — no cross-stream starvation.
- HBM traffic: 2 MiB/core (bf16), single SP HWDGE queue, contiguous
  per-partition runs. Chunk-pair schedule [1,2,2,2,1]: small first chunk
  (early compute start ~receipt-latency bound) and small last chunk
  (short dependency tail).
- Per-row products: DVE scalar_tensor_tensor (fused multiply+accum,
  687 ns/block) and ACT Square-with-accumulate (~800 ns/block effective).
  24 block-products split ACT:11 / DVE:13 to finish together.
- Stats block-major ([xt,xx,tt] per block); blocks 0-6 DMA out early,
  block 7 in a tiny final DMA. Host does cosine + mean in f64.
"""

import sys

for _p in ("/opt/trn_rl_repo",):
    if _p not in sys.path:
        sys.path.insert(0, _p)

import ml_dtypes
import numpy as np

import concourse.tile as tile
from concourse import bacc, mybir
from concourse import bass_utils

N, D = 8192, 512
NCORES = 8
N_LOC = N // NCORES          # 1024 rows per core
P = 128                      # partitions
NT = N_LOC // P              # 8 blocks of [128, 512] per core
CHUNKS = [1, 2, 2, 2, 1]     # block-PAIRS per dma_start ((x,t) both)
OUT_SPLIT = 7                # blocks 0..6 DMA'd out early, block 7 at the end
# ACT: all xx squares + three tt squares; DVE: all xt + remaining tt.
ACT_PRODUCTS = ({("xx", i) for i in range(NT)} | {("tt", i) for i in (1, 4, 6)})

F32 = mybir.dt.float32
BF16 = mybir.dt.bfloat16
MULT = mybir.AluOpType.mult


def _build():
    nc = bacc.Bacc("TRN2", target_bir_lowering=False, debug=False, num_devices=NCORES)
    xt_in = nc.dram_tensor("xt", [P, 2 * NT * D], BF16, kind="ExternalInput").ap()
    # block-major per-row stats: col 3i+0 = <x,t>, 3i+1 = <x,x>, 3i+2 = <t,t>
    out = nc.dram_tensor("dots", [P, 3 * NT], F32, kind="ExternalOutput").ap()
    off = {"xt": 0, "xx": 1, "tt": 2}

    assert sum(CHUNKS) == NT

    with tile.TileContext(nc) as tc:
        with (
            tc.tile_pool(name="inp", bufs=len(CHUNKS)) as inpool,
            tc.tile_pool(name="scratch", bufs=6) as spool,
            tc.tile_pool(name="stats", bufs=1) as stats,
        ):
            dots = stats.tile([P, 3 * NT], F32, tag="dots")
            warm = stats.tile([P, 1], BF16, tag="warm")
            # ACT warmup: ACT_TABLE_LOAD overlaps the DMA wait
            one_bf16 = nc.const_aps.aps[(BF16, 1.0)]
            nc.scalar.activation(warm[:], one_bf16, mybir.ActivationFunctionType.Square)

            def acc_ap(stat, i):
                c = 3 * i + off[stat]
                return dots[:, c : c + 1]

            def act_square(src, stat, i):
                sq = spool.tile([P, D], BF16, tag="sq")
                nc.scalar.activation(
                    sq[:], src, mybir.ActivationFunctionType.Square,
                    accum_out=acc_ap(stat, i),
                )

            def dve_stt(a, b, stat, i):
                pr = spool.tile([P, D], BF16, tag="pr")
                nc.vector.scalar_tensor_tensor(
                    pr[:], a, 1.0, b, op0=MULT, op1=MULT,
                    accum_out=acc_ap(stat, i),
                )

            base = 0
            for sz in CHUNKS:
                tin = inpool.tile([P, sz * 2 * D], BF16, tag="pair")
                nc.sync.dma_start(
                    tin[:], xt_in[:, base * 2 * D : (base + sz) * 2 * D]
                )
                for j in range(sz):
                    i = base + j
                    xa = tin[:, (2 * j) * D : (2 * j + 1) * D]
                    ta = tin[:, (2 * j + 1) * D : (2 * j + 2) * D]
                    act_square(xa, "xx", i)
                    if ("tt", i) in ACT_PRODUCTS:
                        act_square(ta, "tt", i)
                    else:
                        dve_stt(ta, ta, "tt", i)
                    dve_stt(xa, ta, "xt", i)
                base += sz

            # early out-DMA for blocks 0..OUT_SPLIT-1, tiny final one
            nc.sync.dma_start(out[:, : 3 * OUT_SPLIT], dots[:, : 3 * OUT_SPLIT])
            nc.sync.dma_start(out[:, 3 * OUT_SPLIT :], dots[:, 3 * OUT_SPLIT :])

    nc.finalize()
    return nc


_nc_cache = None


def _get_nc():
    global _nc_cache
    if _nc_cache is None:
        _nc_cache = _build()
    return _nc_cache


def run(x, x_target, **spmd_kwargs):
    """Run the SPMD kernel; returns (loss, BassKernelResults)."""
    x = np.asarray(x, dtype=np.float32).astype(ml_dtypes.bfloat16)
    t = np.asarray(x_target, dtype=np.float32).astype(ml_dtypes.bfloat16)
    assert x.shape == (N, D) and t.shape == (N, D)
    nc = _get_nc()
    in_maps = []
    for c in range(NCORES):
        xs = x[c * N_LOC : (c + 1) * N_LOC].reshape(P, NT, 1, D)
        ts = t[c * N_LOC : (c + 1) * N_LOC].reshape(P, NT, 1, D)
        # interleave: [P, NT, 2, D] -> x at even block-slots, t at odd
        pair = np.concatenate([xs, ts], axis=2).reshape(P, 2 * NT * D)
        in_maps.append({"xt": np.ascontiguousarray(pair)})
    res = bass_utils.run_bass_kernel_spmd(
        nc, in_maps, core_ids=list(range(NCORES)), **spmd_kwargs
    )
    dots = np.stack([np.asarray(r["dots"]) for r in res.results]).astype(np.float64)
    dots = dots.reshape(NCORES, P, NT, 3)
    xt = dots[..., 0]
    xx = dots[..., 1]
    tt = dots[..., 2]
    EPS = 1e-8  # matches reference: a / max(||a||, eps) per tensor
    cos = xt / (np.maximum(np.sqrt(xx), EPS) * np.maximum(np.sqrt(tt), EPS))
    loss = 2.0 - 2.0 * float(np.mean(cos))
    return np.float32(loss), res


def kernel(x, x_target):
    loss, _ = run(x, x_target)
    return loss


# revision 11
# speedup vs baseline: 1.0506x; 1.0506x over previous
"""BYOL loss kernel for Trainium2 (8 NeuronCores, SPMD data-parallel).

loss = 2 - 2 * mean_n( <x_n, t_n> / (||x_n|| * ||t_n||) )   over N=8192 rows, D=512.

v6 design (HW-measured op costs):
- Host casts f32 -> bf16, reshapes each core's [1024, 512] shards and
  interleaves x/t per 128-row block into ONE [128, 2*4096] DRAM tensor:
  row r = p*8 + slot on partition p; block i's x at free-offset 2i*512,
  t at (2i+1)*512. Every DMA chunk delivers complete (x,t) block pairs,
  so each DMA-completion semaphore unlocks all 3 products of its blocks
  (xt, xx, tt) for both compute engines — no cross-stream starvation.
- HBM traffic: 2 MiB/core (bf16), single SP HWDGE queue, contiguous
  per-partition runs. Chunk-pair schedule [1,2,2,2,1]: small first chunk
  (early compute start ~receipt-latency bound) and small last chunk
  (short dependency tail).
- Per-row products: DVE scalar_tensor_tensor (fused multiply+accum,
  687 ns/block) and ACT Square-with-accumulate (~800 ns/block effective).
  24 block-products split ACT:11 / DVE:13 to finish together.
- Stats block-major ([xt,xx,tt] per block); blocks 0-6 DMA out early,
  block 7 in a tiny final DMA. Host does cosine + mean in f64.
"""

import sys

for _p in ("/opt/trn_rl_repo",):
    if _p not in sys.path:
        sys.path.insert(0, _p)

import ml_dtypes
import numpy as np

import concourse.tile as tile
from concourse import bacc, mybir
from concourse import bass_utils

N, D = 8192, 512
NCORES = 8
N_LOC = N // NCORES          # 1024 rows per core
P = 128                      # partitions
NT = N_LOC // P              # 8 blocks of [128, 512] per core
CHUNKS = [1] * 8             # block-PAIRS per dma_start ((x,t) both)
OUT_SPLIT = 7                # blocks 0..6 DMA'd out early, block 7 at the end
# ACT: all xx squares + three tt squares; DVE: all xt + remaining tt.
ACT_PRODUCTS = ({("xx", i) for i in range(NT)} | {("tt", i) for i in (1, 4, 6)})

F32 = mybir.dt.float32
BF16 = mybir.dt.bfloat16
MULT = mybir.AluOpType.mult


def _build():
    nc = bacc.Bacc("TRN2", target_bir_lowering=False, debug=False, num_devices=NCORES)
    xt_in = nc.dram_tensor("xt", [P, 2 * NT * D], BF16, kind="ExternalInput").ap()
    # block-major per-row stats: col 3i+0 = <x,t>, 3i+1 = <x,x>, 3i+2 = <t,t>
    out = nc.dram_tensor("dots", [P, 3 * NT], F32, kind="ExternalOutput").ap()
    off = {"xt": 0, "xx": 1, "tt": 2}

    assert sum(CHUNKS) == NT

    with tile.TileContext(nc) as tc:
        with (
            tc.tile_pool(name="inp", bufs=len(CHUNKS)) as inpool,
            tc.tile_pool(name="scratch", bufs=6) as spool,
            tc.tile_pool(name="stats", bufs=1) as stats,
        ):
            dots = stats.tile([P, 3 * NT], F32, tag="dots")
            warm = stats.tile([P, 1], BF16, tag="warm")
            # ACT warmup: ACT_TABLE_LOAD overlaps the DMA wait
            one_bf16 = nc.const_aps.aps[(BF16, 1.0)]
            nc.scalar.activation(warm[:], one_bf16, mybir.ActivationFunctionType.Square)

            def acc_ap(stat, i):
                c = 3 * i + off[stat]
                return dots[:, c : c + 1]

            def act_square(src, stat, i):
                sq = spool.tile([P, D], BF16, tag="sq")
                nc.scalar.activation(
                    sq[:], src, mybir.ActivationFunctionType.Square,
                    accum_out=acc_ap(stat, i),
                )

            def dve_stt(a, b, stat, i):
                pr = spool.tile([P, D], BF16, tag="pr")
                nc.vector.scalar_tensor_tensor(
                    pr[:], a, 1.0, b, op0=MULT, op1=MULT,
                    accum_out=acc_ap(stat, i),
                )

            base = 0
            for sz in CHUNKS:
                tin = inpool.tile([P, sz * 2 * D], BF16, tag="pair")
                nc.sync.dma_start(
                    tin[:], xt_in[:, base * 2 * D : (base + sz) * 2 * D]
                )
                for j in range(sz):
                    i = base + j
                    xa = tin[:, (2 * j) * D : (2 * j + 1) * D]
                    ta = tin[:, (2 * j + 1) * D : (2 * j + 2) * D]
                    act_square(xa, "xx", i)
                    if ("tt", i) in ACT_PRODUCTS:
                        act_square(ta, "tt", i)
                    else:
                        dve_stt(ta, ta, "tt", i)
                    dve_stt(xa, ta, "xt", i)
                base += sz

            # early out-DMA for blocks 0..OUT_SPLIT-1, tiny final one
            nc.sync.dma_start(out[:, : 3 * OUT_SPLIT], dots[:, : 3 * OUT_SPLIT])
            nc.sync.dma_start(out[:, 3 * OUT_SPLIT :], dots[:, 3 * OUT_SPLIT :])

    nc.finalize()
    return nc


_nc_cache = None


def _get_nc():
    global _nc_cache
    if _nc_cache is None:
        _nc_cache = _build()
    return _nc_cache


def run(x, x_target, **spmd_kwargs):
    """Run the SPMD kernel; returns (loss, BassKernelResults)."""
    x = np.asarray(x, dtype=np.float32).astype(ml_dtypes.bfloat16)
    t = np.asarray(x_target, dtype=np.float32).astype(ml_dtypes.bfloat16)
    assert x.shape == (N, D) and t.shape == (N, D)
    nc = _get_nc()
    in_maps = []
    for c in range(NCORES):
        xs = x[c * N_LOC : (c + 1) * N_LOC].reshape(P, NT, 1, D)
        ts = t[c * N_LOC : (c + 1) * N_LOC].reshape(P, NT, 1, D)
        # interleave: [P, NT, 2, D] -> x at even block-slots, t at odd
        pair = np.concatenate([xs, ts], axis=2).reshape(P, 2 * NT * D)
        in_maps.append({"xt": np.ascontiguousarray(pair)})
    res = bass_utils.run_bass_kernel_spmd(
        nc, in_maps, core_ids=list(range(NCORES)), **spmd_kwargs
    )
    dots = np.stack([np.asarray(r["dots"]) for r in res.results]).astype(np.float64)
    dots = dots.reshape(NCORES, P, NT, 3)
    xt = dots[..., 0]
    xx = dots[..., 1]
    tt = dots[..., 2]
    EPS = 1e-8  # matches reference: a / max(||a||, eps) per tensor
    cos = xt / (np.maximum(np.sqrt(xx), EPS) * np.maximum(np.sqrt(tt), EPS))
    loss = 2.0 - 2.0 * float(np.mean(cos))
    return np.float32(loss), res


def kernel(x, x_target):
    loss, _ = run(x, x_target)
    return loss
